# revision 3
# baseline (speedup 1.0000x reference)
"""Equivariant-subsample (shifted 2x2 max-pool) Trainium2 kernel.

Problem: images [16,64,512,512] f32, per-(b,c) offsets p_h, p_w in {0,1}.
out[b,c,i,j] = max over rows {rA, rA+1} x cols {cA, cA+1} of images[b,c]
where rA = min(2*i + p_h, 510), cA = min(2*j + p_w, 510).

Strategy (8 NeuronCores, embarrassingly data-parallel):
  - Flatten (b,c) -> 1024 images; core k owns images [k*128, (k+1)*128).
    One image per SBUF partition.
  - Key observation: the two source rows of every output row are ADJACENT
    (rowB = rowA + 1 even at the clamp), so a single gather index per
    output row fetches a contiguous 1024-element (4 KB) segment covering
    both rows, with the p_w column shift folded into the element offset.
    Indices are computed on the host from p_h/p_w (tiny metadata, like DMA
    descriptors) and uploaded as an int32 tensor; the compiled program is
    input-independent.
  - On device: indirect DMA gather (SWDGE) -> 4-way strided tensor_tensor
    max on DVE (a pure 2x2 maxpool) -> store.  The only place the p_w
    clamp deviates from the uniform stride is output column 255; a tiny
    per-partition blend (biases uploaded from host) fixes it.
  - Out-of-bounds tail: a segment of the last row with p_w=1 spills 1
    element past the image.  Cores 0-6 use an input view overlapping the
    next core's first row (zero-copy); core 7 gets a 2 KB zero pad.
  - Pipelining: ld ring depth 4, oc ring depth 4.  The WAR wait for an oc
    slot (store 4 chunks back) attaches to the first oc write, which sits
    AFTER the ld readers in the in-order DVE stream, so a slow store can
    never stall the gather pipeline.  The tail tapers to 4/2/1-row chunks
    and output row 255 ships early from the edge-pair block, so the drain
    tail (last gather -> serial DVE -> final store) is tiny.  Stores
    alternate between the two HWDGE queues (SP / Activation), each served
    by its own subset of DMA engines.
"""

import sys

import numpy as np

sys.path.insert(0, "/opt/trn_rl_repo")

B, C, H, W = 16, 64, 512, 512
HR = WR = 2
OH, OW = H // HR, W // WR
NCORES = 8
P = 128                     # SBUF partitions == images per core
IMGS = (B * C) // NCORES    # 128
RC = 8                      # output rows per full chunk
NFULL = OH // RC - 1        # 31 full chunks
# Tapered tail: output rows 248..254 come from 4/2/1-row chunks so the
# drain tail (last gather -> serial DVE -> final store) is tiny; output
# row 255 is stored directly from the edge-pair block, which runs at the
# very start of the kernel.
TAPER = (4, 2, 1)
NIDX = NFULL + len(TAPER) + 1   # full + taper + 1 edge-pair index
NROWS_PAD = IMGS * H + 2    # input rows per core incl. 2 pad rows
NEG = np.float32(-3.0e38)

_prog = None


def _legalize_waits(nc, mybir, dummy_sem_id, dummy_sem_name):
    """Split multi-wait instructions: this walrus build encodes only ONE
    sync-wait per engine/DMA instruction.  Hoist extra waits onto no-op
    instructions inserted just before, on the same engine (the sequencer
    executes them in order, so the AND-semantics are preserved)."""
    for fn in nc.m.functions:
        for blk in fn.blocks:
            new_insts = []
            for inst in blk.instructions:
                si = getattr(inst, "sync_info", None)
                if si is not None and si.on_wait and len(si.on_wait) > 1:
                    for w in si.on_wait[:-1]:
                        nop = mybir.InstNoOp(
                            name=nc.get_next_instruction_name(),
                            engine=inst.engine,
                            text_hint="wait_split",
                            bass_nofuse=True,
                        )
                        # +1 update on a dedicated, never-waited semaphore
                        # keeps the race detector and ISA checks happy
                        # without perturbing any real threshold.
                        nop.sync_info = mybir.SyncInfo(
                            on_wait=[w],
                            on_update=[
                                mybir.SyncUpdate(
                                    sync_type="semaphore",
                                    id=dummy_sem_id,
                                    update_mode="sem-inc",
                                    ant_name=dummy_sem_name,
                                    update_value=1,
                                )
                            ],
                        )
                        new_insts.append(nop)
                    si.on_wait = si.on_wait[-1:]
                new_insts.append(inst)
            blk.instructions = new_insts


def _build_program():
    from concourse import bass, mybir
    import concourse.tile as tile

    f32 = mybir.dt.float32
    i32 = mybir.dt.int32

    nc = bass.Bass()
    legal_sem = nc.alloc_semaphore("legalize_nop")
    img = nc.declare_dram_parameter("img", [NROWS_PAD, W], f32, isOutput=False)
    # idx[:, c]: one gather index per (partition, chunk) — each partition's
    # chunk of 2*rc input rows is contiguous in DRAM, so one 32 KB (16 KB
    # for the half chunks) descriptor per partition replaces 8 4 KB ones.
    # idx[:, NIDX-1]: the (510,511) edge-pair index.
    idx = nc.declare_dram_parameter("idx", [P, NIDX], i32, isOutput=False)
    bias = nc.declare_dram_parameter("bias", [P, 2], f32, isOutput=False)
    out = nc.declare_dram_parameter("out", [P, OH * OW], f32, isOutput=True)

    # chunk schedule: (idx column, output-row offset, rows in chunk)
    chunks = [(c, c * RC, RC) for c in range(NFULL)]
    orow = NFULL * RC
    for t, rc in enumerate(TAPER):
        chunks.append((NFULL + t, orow, rc))
        orow += rc

    with tile.TileContext(nc) as tc:
        with (
            tc.tile_pool(name="const", bufs=1) as cpool,
            tc.tile_pool(name="ld", bufs=1) as ldpool,
            tc.tile_pool(name="work", bufs=1) as wpool,
            tc.tile_pool(name="res", bufs=1) as rpool,
        ):
            # idx rides the gpsimd (SWDGE) queue: the Pool engine comes up
            # ~1.3us before SP, and the first gather (also on gpsimd) can
            # follow it immediately.  bias goes via the scalar HWDGE queue
            # so it never delays the first gather's descriptor gen.
            idx_sb = cpool.tile([P, NIDX], i32)
            nc.gpsimd.dma_start(out=idx_sb[:], in_=idx[:])
            bias_sb = cpool.tile([P, 2], f32)
            nc.scalar.dma_start(out=bias_sb[:], in_=bias[:])

            ea = cpool.tile([P, OW], f32)

            def emit_edge_block():
                # Output row 255 is parity-independent (always source rows
                # 510/511): compute it from a dedicated one-index-per-
                # partition gather of that row pair, shifted by pw.  Emitted
                # after chunk 0 so it never delays the first main gather.
                et = cpool.tile([P, 2 * W], f32)
                nc.gpsimd.indirect_dma_start(
                    out=et[:],
                    out_offset=None,
                    in_=img[:],
                    in_offset=bass.IndirectOffsetOnAxis(
                        ap=idx_sb[:, NIDX - 1:NIDX], axis=1
                    ),
                )
                ev = et[:].rearrange("p (a j e) -> p a j e", a=2, j=OW, e=2)
                eu1 = cpool.tile([P, OW], f32)
                eu2 = cpool.tile([P, OW], f32)
                nc.vector.tensor_tensor(
                    out=eu1[:], in0=ev[:, 0, :, 0], in1=ev[:, 1, :, 0],
                    op=mybir.AluOpType.max,
                )
                nc.vector.tensor_tensor(
                    out=eu2[:], in0=ev[:, 0, :, 1], in1=ev[:, 1, :, 1],
                    op=mybir.AluOpType.max,
                )
                nc.vector.tensor_tensor(
                    out=ea[:], in0=eu1[:], in1=eu2[:], op=mybir.AluOpType.max,
                )
                ew = et[:].rearrange("p (a w) -> p a w", a=2, w=W)
                ee2 = cpool.tile([P, 2], f32)
                nc.vector.tensor_tensor(
                    out=ee2[:], in0=ew[:, 0, 509:511], in1=ew[:, 1, 509:511],
                    op=mybir.AluOpType.max,
                )
                efx = cpool.tile([P, 1], f32)
                nc.vector.tensor_tensor(
                    out=efx[:], in0=ee2[:, 0:1], in1=ee2[:, 1:2],
                    op=mybir.AluOpType.max,
                )
                eta = cpool.tile([P, 1], f32)
                etb = cpool.tile([P, 1], f32)
                nc.vector.tensor_tensor(
                    out=eta[:], in0=ea[:, 255:256], in1=bias_sb[:, 0:1],
                    op=mybir.AluOpType.add,
                )
                nc.vector.tensor_tensor(
                    out=etb[:], in0=efx[:], in1=bias_sb[:, 1:2],
                    op=mybir.AluOpType.add,
                )
                nc.vector.tensor_tensor(
                    out=ea[:, 255:256], in0=eta[:], in1=etb[:],
                    op=mybir.AluOpType.max,
                )
                # Output row 255 ships directly from here, ~20us into the
                # kernel — nothing about it remains in the drain tail.
                nc.scalar.dma_start(
                    out=out[:, (OH - 1) * OW:OH * OW], in_=ea[:]
                )

            def emit_gather(seq, col, rc):
                # Explicit modular tags force true round-robin slot reuse
                # (ld/oc ring depth 4).  Gather the chunk: 2*rc contiguous
                # input rows per partition, shifted by (ph, pw) via the
                # per-partition index.
                ld = ldpool.tile(
                    [P, rc * 2 * W], f32, tag=f"ld{seq % 4}",
                    padded_shape=[P, RC * 2 * W], name=f"ld_{seq}",
                )
                ld_inst = nc.gpsimd.indirect_dma_start(
                    out=ld[:],
                    out_offset=None,
                    in_=img[:],
                    in_offset=bass.IndirectOffsetOnAxis(
                        ap=idx_sb[:, col:col + 1], axis=1
                    ),
                )
                # One packet per 32 KB descriptor: the SDMA engine stalls ~170
                # ns on the HBM read round-trip at every intra-descriptor
                # packet boundary when its queue is the only one with work
                # (observed 12-16 GB/s/engine early vs 27.1 when store queues
                # interleave).  Single-packet descriptors pipeline at line
                # rate descriptor-to-descriptor.
                ld_inst.ins.single_packet = True
                return ld

            def emit_compute(eng, wt, seq, orow, rc, ld):
                # ld[p, ((r*2 + a)*OW + j)*2 + e] = row a of pair r, col 2j+e
                ldv = ld[:].rearrange("p (r a j e) -> p r a j e", a=2, j=OW, e=2)
                # Work temporaries are produced and consumed only by `eng`,
                # which executes in order: one slot per engine (tag prefix
                # `wt`), no cross-engine WAR possible.
                t1 = wpool.tile([P, rc * OW], f32, tag=f"{wt}t1",
                                padded_shape=[P, RC * OW], name=f"t1_{seq}")
                t2 = wpool.tile([P, rc * OW], f32, tag=f"{wt}t2",
                                padded_shape=[P, RC * OW], name=f"t2_{seq}")
                # oc ring depth 4: the WAR wait (store seq-4 done) lands on
                # the first oc writer below, AFTER the ld readers t1/t2/e2,
                # so a late store can never stall the gather pipeline.
                oc = rpool.tile([P, rc * OW], f32, tag=f"oc{seq % 4}",
                                padded_shape=[P, RC * OW], name=f"oc_{seq}")
                t1v = t1[:].rearrange("p (r j) -> p r j", j=OW)
                t2v = t2[:].rearrange("p (r j) -> p r j", j=OW)
                ocv = oc[:].rearrange("p (r j) -> p r j", j=OW)
                # 2x2 max pool: max over row-in-pair (a) and col-in-pair (e)
                eng.tensor_tensor(
                    out=t1v, in0=ldv[:, :, 0, :, 0], in1=ldv[:, :, 1, :, 0],
                    op=mybir.AluOpType.max,
                )
                eng.tensor_tensor(
                    out=t2v, in0=ldv[:, :, 0, :, 1], in1=ldv[:, :, 1, :, 1],
                    op=mybir.AluOpType.max,
                )

                # Output col 255 fix: when pw==1 the correct value is the max
                # over segment positions (509, 510) of both rows (= source
                # cols 510, 511); the uniform stride used (510, 511) instead.
                # e2 runs before the oc write so the ld slot is released as
                # early as possible (it is the last ld reader).
                ldw = ld[:].rearrange("p (r a w) -> p r a w", a=2, w=W)
                e2 = wpool.tile([P, rc * 2], f32, tag=f"{wt}e2",
                                padded_shape=[P, RC * 2], name=f"e2_{seq}")
                e2v = e2[:].rearrange("p (r e) -> p r e", e=2)
                eng.tensor_tensor(
                    out=e2v, in0=ldw[:, :, 0, 509:511], in1=ldw[:, :, 1, 509:511],
                    op=mybir.AluOpType.max,
                )
                eng.tensor_tensor(
                    out=ocv, in0=t1v, in1=t2v, op=mybir.AluOpType.max,
                )
                fx = wpool.tile([P, rc], f32, tag=f"{wt}fx",
                                padded_shape=[P, RC], name=f"fx_{seq}")
                eng.tensor_tensor(
                    out=fx[:], in0=e2v[:, :, 0], in1=e2v[:, :, 1],
                    op=mybir.AluOpType.max,
                )
                ta = wpool.tile([P, rc], f32, tag=f"{wt}ta",
                                padded_shape=[P, RC], name=f"ta_{seq}")
                tb = wpool.tile([P, rc], f32, tag=f"{wt}tb",
                                padded_shape=[P, RC], name=f"tb_{seq}")
                # bias0 = (pw==0 ? 0 : -BIG), bias1 = (pw==0 ? -BIG : 0)
                eng.tensor_tensor(
                    out=ta[:], in0=ocv[:, :, 255],
                    in1=bias_sb[:, 0:1].to_broadcast([P, rc]),
                    op=mybir.AluOpType.add,
                )
                eng.tensor_tensor(
                    out=tb[:], in0=fx[:],
                    in1=bias_sb[:, 1:2].to_broadcast([P, rc]),
                    op=mybir.AluOpType.add,
                )
                eng.tensor_tensor(
                    out=ocv[:, :, 255], in0=ta[:], in1=tb[:],
                    op=mybir.AluOpType.max,
                )

                # Split every store across BOTH HWDGE rings (SP / Act): keeps
                # both rings continuously backlogged so the SDMA engines always
                # have a second queue context to interleave with the gather
                # queue (hides the per-packet HBM read latency on the loads).
                if rc > 1:
                    half = rc // 2
                    nc.sync.dma_start(
                        out=out[:, orow * OW:(orow + half) * OW],
                        in_=oc[:, :half * OW],
                    )
                    nc.scalar.dma_start(
                        out=out[:, (orow + half) * OW:(orow + rc) * OW],
                        in_=oc[:, half * OW:rc * OW],
                    )
                else:
                    seng = nc.sync if seq % 2 == 0 else nc.scalar
                    seng.dma_start(
                        out=out[:, orow * OW:(orow + rc) * OW], in_=oc[:]
                    )

            # Main stream: gather + DVE compute per chunk.
            for seq, (col, orow, rc) in enumerate(chunks[:NFULL]):
                ld = emit_gather(seq, col, rc)
                emit_compute(nc.vector, "", seq, orow, rc, ld)
                if seq == 0:
                    emit_edge_block()

            # Tail taper: issue ALL taper gathers first so their transfers
            # queue back-to-back, then run the computes (all on DVE —
            # TensorTensor is not a legal Pool-engine opcode on this core).
            taper_lds = [
                emit_gather(NFULL + t, col, rc)
                for t, (col, orow, rc) in enumerate(chunks[NFULL:])
            ]
            for t, (col, orow, rc) in enumerate(chunks[NFULL:]):
                emit_compute(nc.vector, "", NFULL + t, orow, rc, taper_lds[t])
    _legalize_waits(nc, mybir, legal_sem.num, legal_sem.name)
    return nc


def _host_inputs(images, p_w, p_h):
    """Build the 8 per-core input maps (views wherever possible)."""
    flat = np.ascontiguousarray(images, dtype=np.float32).reshape(-1)
    ph = np.asarray(p_h).reshape(-1).astype(np.int64)
    pw = np.asarray(p_w).reshape(-1).astype(np.int64)
    nelem = IMGS * H * W
    in_maps = []
    for k in range(NCORES):
        if k < NCORES - 1:
            img_k = flat[k * nelem:(k + 1) * nelem + 2 * W].reshape(NROWS_PAD, W)
        else:
            img_k = np.concatenate(
                [flat[k * nelem:], np.zeros(2 * W, np.float32)]
            ).reshape(NROWS_PAD, W)
        phk = ph[k * IMGS:(k + 1) * IMGS]
        pwk = pw[k * IMGS:(k + 1) * IMGS]
        # One index per chunk: the chunk's 2*rc input rows are contiguous in
        # DRAM (consecutive pairs are adjacent rows), so each partition's
        # chunk is a single 2*rc*W-element read starting at row r0 + ph,
        # col 0... shifted by pw.  Unclamped: the last (half) chunk of a
        # ph=1 image reads one garbage row; output row 255 is overwritten
        # on-device from the edge-pair gather (extra column).
        base = np.arange(IMGS, dtype=np.int64)[:, None] * H
        taper_starts = []
        r0 = 2 * RC * NFULL
        for rc in TAPER:
            taper_starts.append(r0)
            r0 += 2 * rc
        starts = np.array(
            [2 * RC * c for c in range(NFULL)] + taper_starts,
            dtype=np.int64,
        )
        idx_main = (base + starts[None, :] + phk[:, None]) * W + pwk[:, None]
        idx_edge = (base[:, 0] + H - HR) * W + pwk                   # [IMGS]
        idx = np.concatenate(
            [idx_main, idx_edge[:, None]], axis=1
        ).astype(np.int32)
        bias = np.stack(
            [np.where(pwk == 0, 0.0, NEG), np.where(pwk == 0, NEG, 0.0)],
            axis=1,
        ).astype(np.float32)
        in_maps.append({"img": img_k, "idx": idx, "bias": bias})
    return in_maps


def _get_prog():
    global _prog
    if _prog is None:
        _prog = _build_program()
    return _prog


def kernel(images, p_w, p_h, _return_raw=False, **run_kwargs):
    from concourse.bass_utils import run_bass_kernel_spmd

    in_maps = _host_inputs(images, p_w, p_h)
    res = run_bass_kernel_spmd(
        _get_prog(), in_maps, list(range(NCORES)), **run_kwargs
    )
    outs = [r["out"].reshape(IMGS, OH, OW) for r in res.results]
    full = np.concatenate(outs, axis=0).reshape(B, C, OH, OW)
    if _return_raw:
        return full, res
    return full

